# revision 1
# baseline (speedup 1.0000x reference)
"""Boundary-loss kernel for Trainium2 (8 NeuronCores, pure data parallel).

Computes mean(phi_G * sigmoid(predictions)) where phi_G is the per-sample
normalized signed EDT of the target mask, via phi = (1-2t) * u2d with
u2d = distance to the nearest opposite-class pixel:

    u2d(r,c)^2 = min( (ucol+1)^2,  min_{|k|<=K} (hrow(r+k,c)+1)^2 + k^2 )

hrow / ucol are exact 1-D opposite-distances (minus 1) along rows /
columns, from two tensor_tensor_scan passes over an equality field.
Only the vertical parabola is windowed (K), certified by max(E) <= K^2.

Engine split: DVE does the scans/mins, ACT does sigmoid/squares/adds/sqrt
(Square and Copy are resident in both loaded table sets), PE transposes
hrow into the column layout, the DMA xbar transposes the mask and the
signed sigmoid, GPSIMD does backward scans and memsets.  Output per core
is [128, 2] (partial sum, partial max); host normalizes and reduces.
"""

import numpy as np
from contextlib import ExitStack

import concourse.bass as bass
import concourse.tile as tile
from concourse import bacc, mybir, masks
from concourse.bass_utils import run_bass_kernel_spmd

B, C, H, W = 8, 1, 256, 256
P = 128
NCH = 2
BIG = 300.0
PADV = 60000.0
YW = 2 * 256 + 1          # 513

Alu = mybir.AluOpType
Act = mybir.ActivationFunctionType
F32 = mybir.dt.float32
BF16 = mybir.dt.bfloat16
I32 = mybir.dt.int32

_K_LADDER = [3, 7, 15, 31, 63, 127, 255]
GPSIMD_SCANS = False      # gpsimd scans do not compile in this toolchain


def _seg2(ap_tile, start, segstr, width=256):
    return (ap_tile[:, start:start + 2 * segstr]
            .rearrange("p (s t) -> p s t", s=2)[:, :, 0:width])


def _emit_chain(nc, pool, tag, Ysrc0, Ysrc1, ONEB, U_out, merge_split):
    """Exact 1-D opposite-distance minus 1 along the free dim.

    state = (1 + state) * y with y = [t(j)==t(j+1)]; an edge (y=0) resets
    the counter.  ONEB carries BIG at the blocker column to isolate the
    two segments; init=BIG marks "no edge yet".  u-1 = min(F(i-1), G(i)).
    """
    Y = pool.tile([P, YW], BF16, tag=f"Y{tag}")
    nc.gpsimd.memset(Y[:], 1.0)
    nc.vector.tensor_tensor(_seg2(Y, 1, 256, 255), Ysrc0, Ysrc1,
                            op=Alu.is_equal)
    F = pool.tile([P, YW], BF16, tag=f"F{tag}")
    BT = pool.tile([P, YW + 1], BF16, tag=f"B{tag}")
    nc.vector.tensor_tensor_scan(
        out=F[:, 0:YW], data0=ONEB[:, 0:YW], data1=Y[:, 0:YW],
        initial=BIG, op0=Alu.add, op1=Alu.mult)
    beng = nc.gpsimd if GPSIMD_SCANS else nc.vector
    beng.tensor_tensor_scan(
        out=BT[:, YW:0:-1], data0=ONEB[:, YW - 1::-1], data1=Y[:, YW - 1::-1],
        initial=BIG, op0=Alu.add, op1=Alu.mult)
    if merge_split:
        for c in range(NCH):
            nc.vector.tensor_tensor(
                U_out[:, c, :], F[:, 256 * c:256 * c + 256],
                BT[:, 2 + 256 * c:2 + 256 * c + 256], op=Alu.min)
    else:
        nc.vector.tensor_tensor(
            U_out[:], _seg2(F, 0, 256), _seg2(BT, 2, 256), op=Alu.min)


def _kernel_body(ctx: ExitStack, tc, out_ap, tgt_ap, pred_ap, K: int):
    nc = tc.nc
    use_bf16 = K <= 11
    dt_e = BF16 if use_bf16 else F32

    SEGSTR = 256 + 2 * K + 2
    LP = K + 1
    PW = LP + 2 * SEGSTR + K + 2

    pool = ctx.enter_context(tc.tile_pool(name="work", bufs=1))
    psum = ctx.enter_context(tc.tile_pool(name="ps", bufs=1, space="PSUM"))

    # ---------------- input DMAs ----------------
    T = pool.tile([P, NCH, 256], I32, tag="T")
    Pt = pool.tile([P, NCH, 256], F32, tag="Pt")
    nc.sync.dma_start(T[:], tgt_ap.rearrange("(c p) j -> p c j", p=P))
    nc.scalar.dma_start(Pt[:], pred_ap.rearrange("(c p) j -> p c j", p=P))

    # ---------------- constants ----------------
    dummy1 = pool.tile([1, 8], F32, tag="dm1")
    dummy2 = pool.tile([1, 8], F32, tag="dm2")
    nc.gpsimd.memset(dummy1[:], 0.5)
    ONEB = pool.tile([P, YW], BF16, tag="ONEB")
    nc.gpsimd.memset(ONEB[:], 1.0)
    nc.gpsimd.memset(ONEB[:, 256:257], BIG)
    p2 = pool.tile([P, PW], dt_e, tag="p2")
    nc.gpsimd.memset(p2[:], PADV)
    ident = pool.tile([P, P], BF16, tag="ident")
    masks.make_identity(nc, ident[:])

    # ---------------- ACT: sigmoid table + casts ----------------
    nc.scalar.activation(dummy2[:], dummy1[:], Act.Sigmoid)
    tb = pool.tile([P, NCH, 256], BF16, tag="tb")
    for c in range(NCH):
        nc.scalar.activation(tb[:, c, :], T[:, c, :], Act.Copy)  # i32 -> bf16
    sg = pool.tile([P, NCH, 256], BF16, tag="sg")
    nc.scalar.activation(sg[:], Pt[:], Act.Sigmoid)

    # ---------------- chain A: horizontal (row layout, on i32) --------
    U = pool.tile([P, NCH, 256], BF16, tag="U")
    _emit_chain(nc, pool, "a", T[:, :, 0:255], T[:, :, 1:256], ONEB, U[:],
                merge_split=True)

    # ---------------- transposed mask via DMA xbar (col = w*128+p) ----
    # split the two transposes across both HWDGE queues so their issue
    # slots (~1.3us each) overlap instead of serializing on sync
    tbT = pool.tile([P, NCH, 256], BF16, tag="tbT")
    nc.sync.dma_start_transpose(tbT[:, :, 0:128], tb[:, 0, :])
    nc.scalar.dma_start_transpose(tbT[:, :, 128:256], tb[:, 1, :])

    # ---------------- chain B: vertical (transposed layout) -----------
    UC = pool.tile([P, NCH, 256], BF16, tag="UC")
    _emit_chain(nc, pool, "b", tbT[:, :, 0:255], tbT[:, :, 1:256], ONEB,
                UC[:], merge_split=False)

    # ---------------- hrow transpose on PE (col = w*128+p) ------------
    uT_ps = psum.tile([P, NCH, 256], BF16, tag="uT_ps")
    for c in range(NCH):
        for w in range(NCH):
            nc.tensor.transpose(uT_ps[:, w, 128 * c:128 * (c + 1)],
                                U[:, c, 128 * w:128 * (w + 1)], ident[:])

    # ---------------- sigma' = (1-2t)*sigmoid, transposed -------------
    sm = pool.tile([P, NCH, 256], BF16, tag="sm")
    nc.vector.tensor_tensor(sm[:], sg[:], tb[:], op=Alu.mult)
    sp = pool.tile([P, NCH, 256], BF16, tag="sp")
    nc.vector.scalar_tensor_tensor(
        sp[:], sm[:], -2.0, sg[:], op0=Alu.mult, op1=Alu.add)
    spT = pool.tile([P, NCH, 256], BF16, tag="spT")
    for c in range(NCH):
        nc.sync.dma_start_transpose(spT[:, :, 128 * c:128 * (c + 1)], sp[:, c, :])

    # ---------------- E = windowed parabola + ucol^2 ------------------
    p2segs = _seg2(p2, LP, SEGSTR)
    nc.scalar.activation(p2segs, uT_ps[:], Act.Square, bias=1.0, scale=1.0)
    q1 = pool.tile([P, PW], dt_e, tag="q1")
    nc.vector.tensor_scalar_add(q1[:], p2[:], 1.0)
    q4 = pool.tile([P, PW], dt_e, tag="q4")
    nc.scalar.activation(q4[:], p2[:], Act.Copy, bias=4.0, scale=1.0)
    c2 = pool.tile([P, NCH, 256], dt_e, tag="c2")
    nc.scalar.activation(c2[:], UC[:], Act.Square, bias=1.0, scale=1.0)
    # preload sqrt table once c2 exists (kept off the E critical path)
    nc.scalar.activation(dummy2[:], c2[0:1, 0, 0:8], Act.Sqrt)

    # The |k|=K ring can never win when E <= K^2 (hrow >= 1 so the term is
    # >= 1+K^2 > K^2, and the pure-vertical case is covered exactly by the
    # ucol term) -- so a K-1 window + ucol certifies against the same
    # max(E) <= K^2 threshold.  Drop the outermost ring.
    qk = {1: q1, 4: q4}
    if K == 3:
        t1p, t1m = _seg2(q1, LP + 1, SEGSTR), _seg2(q1, LP - 1, SEGSTR)
        t2p, t2m = _seg2(q4, LP + 2, SEGSTR), _seg2(q4, LP - 2, SEGSTR)
        EA = pool.tile([P, NCH, 256], dt_e, tag="EA")
        EB = pool.tile([P, NCH, 256], dt_e, tag="EB")
        E = pool.tile([P, NCH, 256], dt_e, tag="E")
        nc.vector.tensor_tensor(EA[:], p2segs, t1p, op=Alu.min)
        nc.vector.tensor_tensor(EB[:], t1m, t2p, op=Alu.min)
        nc.vector.tensor_tensor(EA[:], EA[:], EB[:], op=Alu.min)
        nc.vector.tensor_tensor(EA[:], EA[:], t2m, op=Alu.min)
        nc.vector.tensor_tensor(E[:], EA[:], c2[:], op=Alu.min)
    else:
        E = pool.tile([P, NCH, 256], dt_e, tag="E")
        first = True
        # terminal K=255 runs uncertified -> must keep the full window
        kmax = K + 1 if K >= 255 else K
        for k in range(1, kmax):       # |k|=K ring provably never wins
            if k * k not in qk:
                qx = pool.tile([P, PW], dt_e, tag=f"q{k * k}")
                nc.vector.tensor_scalar_add(qx[:], p2[:], float(k * k))
                qk[k * k] = qx
            for d in (k, -k):
                view = _seg2(qk[k * k], LP + d, SEGSTR)
                nc.vector.tensor_tensor(E[:], p2segs if first else E[:], view,
                                        op=Alu.min)
                first = False
        nc.vector.tensor_tensor(E[:], E[:], c2[:], op=Alu.min)

    # ---------------- outputs ----------------
    OUT = pool.tile([P, 2], F32, tag="OUT")
    nc.vector.tensor_reduce(OUT[:, 1:2], E[:], axis=mybir.AxisListType.XY,
                            op=Alu.max)
    S = pool.tile([P, NCH, 256], F32, tag="S")
    nc.scalar.sqrt(S[:], E[:])
    dump = pool.tile([P, NCH, 256], BF16, tag="dump")
    nc.vector.scalar_tensor_tensor(
        dump[:], S[:], 0.0, spT[:], op0=Alu.bypass, op1=Alu.mult,
        accum_out=OUT[:, 0:1])
    nc.sync.dma_start(out_ap, OUT[:], single_packet=True)


def build(K: int) -> bass.Bass:
    nc = bacc.Bacc("TRN2", target_bir_lowering=False, debug=False,
                   enable_asserts=False, num_devices=B)
    tgt_d = nc.dram_tensor("targets", [H, W], I32, kind="ExternalInput")
    pred_d = nc.dram_tensor("predictions", [H, W], F32, kind="ExternalInput")
    out_d = nc.dram_tensor("out", [P, 2], F32, kind="ExternalOutput")
    with tile.TileContext(nc) as tc:
        with ExitStack() as ctx:
            _kernel_body(ctx, tc, out_d.ap(), tgt_d.ap(), pred_d.ap(), K)
    nc.compile()
    return nc


_nc_cache: dict = {}
LAST_K = 3


def _run(predictions: np.ndarray, targets: np.ndarray, K: int, trace=False):
    if K not in _nc_cache:
        _nc_cache[K] = build(K)
    nc = _nc_cache[K]
    in_maps = [
        {
            "targets": np.ascontiguousarray(targets[b, 0]),
            "predictions": np.ascontiguousarray(predictions[b, 0]),
        }
        for b in range(B)
    ]
    res = run_bass_kernel_spmd(nc, in_maps, core_ids=list(range(B)), trace=trace)
    outs = np.stack([r["out"] for r in res.results])  # (B, 128, 2)
    return outs, res


def _host_reference_sample(t2d, pred2d):
    """Exact numpy port of the reference for one sample (fallback path)."""
    BIGF = float(H + W)
    m = t2d != 0

    def dist1d_h(feat):
        out = np.empty((H, W), np.float64)
        d = np.full(W, BIGF)
        for i in range(H):
            d = np.where(feat[i], 0.0, d + 1.0)
            out[i] = d
        d = np.full(W, BIGF)
        for i in range(H - 1, -1, -1):
            d = np.where(feat[i], 0.0, d + 1.0)
            out[i] = np.minimum(out[i], d)
        return out

    def edt(feat):
        g = np.minimum(dist1d_h(feat), BIGF)
        g2 = g * g
        j = np.arange(W, dtype=np.float64)
        offs = (j[:, None] - j[None, :]) ** 2
        d2 = np.min(g2[:, None, :] + offs[None, :, :], axis=-1)
        return np.sqrt(d2)

    phi = edt(m) - edt(~m)
    denom = np.abs(phi).max() + 1e-8
    if not m.any():
        return 0.0
    sig = 1.0 / (1.0 + np.exp(-pred2d.astype(np.float64)))
    return float((phi / denom * sig).sum())


def kernel(predictions: np.ndarray, targets: np.ndarray) -> np.ndarray:
    global LAST_K
    predictions = np.asarray(predictions, dtype=np.float32)
    targets = np.asarray(targets, dtype=np.int32)

    fg = targets[:, 0] != 0
    nfg = fg.reshape(B, -1).sum(axis=1)
    has_fg = nfg > 0
    mixed = (nfg > 0) & (nfg < H * W)

    ki = 0
    while True:
        K = _K_LADDER[ki]
        outs, _ = _run(predictions, targets, K)
        ssum = outs[:, :, 0].sum(axis=1, dtype=np.float64)
        maxE = outs[:, :, 1].max(axis=1)
        if K >= 255 or not mixed.any() or maxE[mixed].max() <= K * K:
            break
        need = np.sqrt(float(maxE[mixed].max()))
        ki += 1
        while ki < len(_K_LADDER) - 1 and _K_LADDER[ki] < need:
            ki += 1
    LAST_K = K

    total = 0.0
    for b in range(B):
        if not has_fg[b]:
            continue
        if not mixed[b]:
            total += _host_reference_sample(targets[b, 0], predictions[b, 0])
        else:
            denom = np.sqrt(float(maxE[b])) + 1e-8
            total += ssum[b] / denom
    return np.float32(total / (B * C * H * W))


if __name__ == "__main__":
    pred = np.load("/tmp/pred.npy")
    tgt = np.load("/tmp/tgt.npy")
    val = kernel(predictions=pred, targets=tgt)
    print("kernel loss:", repr(val))

